# revision 42
# baseline (speedup 1.0000x reference)
"""Trainium2 Bass kernel for a 2-layer GraphNetwork (gnn_message_passing).

Strategy (final):
  - 16 graphs across 8 cores (2/core, paired big-with-small); every
    edge's receiver is core-local, so all segment reductions stay
    on-core. [16,128] outputs gathered on host.
  - Edge phase per pair of 128-edge chunks (~1.1us steady state):
      * e1 feat-major (2x FD-256, We1Kx-block stationaries) into its
        own 1-bank psum tile -> ACT relu-evac to fp8 e1bf (the split
        tile keeps the evac off the mg matmuls' dependency cone);
      * e1+e2init edge-major as one FD-384 matmul per chunk into a
        2-bank pair tile (the group stays open for e2);
      * e2 as one fp8e4 DoubleRow matmul per chunk (e1bf stationary,
        host-packed We2 pairs moving);
      * one DVE relu-evac of the whole pair ([2,384]-strided) direct
        to fp8 efsb; one DoubleRow aggregation matmul per pair
        (one-hot stationary, exact in fp8; FD 384 at 0.5 cyc/row).
    All stationaries are uniform 128x128 loads - mixed tiling configs
    re-throttle the PE HAM clock to 1.2 GHz (keep PE the busiest
    engine or the idle-window monitor oscillates K=8<->4).
  - Engine budget per pair: PE ~1.1us > DVE ~0.9 (efsb) > ACT ~0.88
    (e1bf + node evacs). Producers lead consumers by a full iteration:
    [e2(g-1), e1T(g), agg(g-2), mg(g)].
  - Node phase spread over 5 iterations per tile: pagg -> aggsb (ACT,
    1/cnt scale), PE transposes -> fp8 aggT, n1/n2 with DoubleRow
    Win1/Wn2 terms, feat-major FD-2 pools accumulated in a [128,4]
    DVE chain; bf16 final projection reads it directly.
  - Startup: 64-row eftM/We1Kx (rows 35:64 host-zeroed; rows 64:128
    memset once on GpSimd, tile 0 runs 64-row stationaries so nothing
    waits); weights in one [128, 2184] bf16 blob; node tensors are
    SBUF-resident, DMAs woven between the first tiles' input loads.
  - CoreV2/V3 codegen only accepts one semaphore wait per queue
    instruction: excess waits are split onto single-wait NOPs.
"""

import numpy as np
import ml_dtypes

import concourse.bass as bass
import concourse.tile as tile_mod
from concourse import tile
from concourse.bass_utils import run_bass_kernel_spmd
from concourse.vector_clock import ScopedClock

mybir = bass.mybir

N_NODES, N_EDGES, N_GRAPHS = 20000, 320000, 16
F_NODE, F_EDGE, F_GLOB = 64, 32, 16
N_CORES = 8
GPC = N_GRAPHS // N_CORES  # graphs per core = 2

BF16 = mybir.dt.bfloat16
F32 = mybir.dt.float32
FP8 = mybir.dt.float8e4
npbf16 = ml_dtypes.bfloat16
npfp8 = mybir.dt.np(FP8)
DR = mybir.MatmulPerfMode.DoubleRow

FP8_AGG = True  # stage-2 toggle: fp8 DoubleRow aggregation

# ---------------------------------------------------------------------------
# Workaround: CoreV2/V3 codegen rejects instructions carrying more than one
# semaphore wait (and the DMA-transpose XPOSE instruction can carry none).
# Split excess waits across single-wait NOPs issued just before on the same
# queue.
_MAX_WAITS = 1


def _split_excess_waits(nc):
    ET = mybir.EngineType
    split_engines = {ET.PE, ET.Activation, ET.DVE, ET.SP, ET.Pool}
    ctr = [0]
    for bass_bb in nc.bb_map.values():
        bb = bass_bb.bb
        out = []
        changed = False
        for inst in bb.instructions:
            si = inst.sync_info
            waits = list(si.on_wait) if (si and si.on_wait) else []
            limit = 0 if isinstance(inst, mybir.InstDmaTransposeAnt) else 1
            if len(waits) > limit and inst.engine in split_engines:
                head = waits[: len(waits) - limit]
                keep = waits[len(waits) - limit:]
                for w in head:
                    nop = mybir.InstNoOp(name=f"waitsplit-{ctr[0]}", ins=[], outs=[])
                    ctr[0] += 1
                    nop.engine = inst.engine
                    nop.sync_info = mybir.SyncInfo(on_wait=[w], on_update=[])
                    nc.register_instruction(nop, overwrite=True)
                    out.append(nop)
                inst.sync_info = mybir.SyncInfo(
                    on_wait=keep, on_update=list(si.on_update or [])
                )
                changed = True
            out.append(inst)
        if changed:
            bb.instructions = out


def _split_drain_and_barrier(self, tick_clock, wait_clock):
    nc = self.nc
    _split_excess_waits(nc)
    drain_inst = nc.sync.drain()
    wait_clock.add_sem_waits(
        drain_inst.ins, ScopedClock({None: tick_clock.global_clock})
    )
    mi = drain_inst.ins
    waits = list(mi.sync_info.on_wait) if (mi.sync_info and mi.sync_info.on_wait) else []
    if len(waits) > _MAX_WAITS:
        upd = list(mi.sync_info.on_update) if mi.sync_info.on_update else []
        mi.sync_info = mybir.SyncInfo(on_wait=waits[:_MAX_WAITS], on_update=upd)
        for i in range(_MAX_WAITS, len(waits), _MAX_WAITS):
            nop = nc.sync.nop(nofuse=True)
            nop.ins.sync_info = mybir.SyncInfo(
                on_wait=waits[i : i + _MAX_WAITS], on_update=[]
            )
    nc.all_engine_barrier()
    assert self.sems is not None
    popped = nc._tile_sem_poison_stack.pop()
    assert popped is self._sem_poison
    nc.clear_and_free_semaphores(list(self.sems.allocated().values()))
    nc.all_engine_barrier()


tile_mod.TileContext._drain_and_barrier = _split_drain_and_barrier

# ---------------------------------------------------------------------------
# The walrus invocation hardcodes --enable-ldw-opt=false; the LDWEIGHTS
# stream is a bottleneck for this kernel, so turn the optimization on.
import concourse.bass_utils as _bu

_orig_run_command = _bu.run_command


def _run_command_ldwopt(argv, **kwargs):
    argv = [
        a
        if isinstance(a, str) else a
        for a in argv
    ]
    return _orig_run_command(argv, **kwargs)


_bu.run_command = _run_command_ldwopt


# ---------------------------------------------------------------------------
# Host-side graph partitioning / layout


def _pack_core(node_ids, degs, nt, cap_e):
    order = np.argsort(-degs, kind="stable")
    tiles_n = [[] for _ in range(nt)]
    tile_ncnt = np.zeros(nt, np.int64)
    tile_ecnt = np.zeros(nt, np.int64)
    for j in order:
        cand = np.where(tile_ncnt < 128)[0]
        if len(cand) == 0:
            return None
        t = cand[np.argmin(tile_ecnt[cand])]
        tiles_n[t].append(node_ids[j])
        tile_ncnt[t] += 1
        tile_ecnt[t] += degs[j]
    if (tile_ecnt > cap_e).any():
        return None
    return [np.array(t, dtype=np.int64) for t in tiles_n]


# weight blob column layout (bf16, 128 rows)
_BLOB_COLS = {
    "We2DR": (0, 256),
    "Wn1TK": (256, 256),
    "Win1DR": (512, 512),
    "Wn2DR": (1024, 256),
    "Win2": (1280, 128),
    "gnaugK": (1408, 128),
    "WgnT": (1536, 128),
    "WgeT": (1664, 128),
    "WggT": (1792, 128),
    "bgr": (1920, 128),
    "ident2": (2048, 2),
    "globT": (2050, 2),
    "ones2": (2052, 2),
    "ident": (2056, 128),
}
_BLOB_W = 2184


def _prepare(inputs):
    nf = np.asarray(inputs["node_feats"], np.float32)
    ef = np.asarray(inputs["edge_feats"], np.float32)
    glob = np.asarray(inputs["globals_"], np.float32)
    recv = np.asarray(inputs["receivers"]).astype(np.int64)
    ngraph = np.asarray(inputs["node_graph"]).astype(np.int64)

    cnt = np.bincount(recv, minlength=N_NODES).astype(np.int64)
    egraph = ngraph[recv]
    ncnt_g = np.bincount(ngraph, minlength=N_GRAPHS)
    ecnt_g = np.bincount(egraph, minlength=N_GRAPHS)

    # pair heavy graphs with light ones to balance nodes across cores
    order = np.argsort(ncnt_g, kind="stable")
    graph_core = np.zeros(N_GRAPHS, np.int64)
    graph_slot = np.zeros(N_GRAPHS, np.int64)
    core_graphs = []
    for c in range(N_CORES):
        ga, gb = int(order[c]), int(order[N_GRAPHS - 1 - c])
        graph_core[ga] = c
        graph_slot[ga] = 0
        graph_core[gb] = c
        graph_slot[gb] = 1
        core_graphs.append((ga, gb))

    node_core = graph_core[ngraph]
    edge_core = graph_core[egraph]

    core_nodes = [np.where(node_core == c)[0] for c in range(N_CORES)]
    NT = int(max((len(cn) + 127) // 128 for cn in core_nodes))

    packs = None
    K0 = max(1, int(max(np.bincount(edge_core, minlength=N_CORES)) + NT * 128 - 1)
             // (NT * 128))
    if K0 % 2:
        K0 += 1
    for k0 in range(K0, K0 + 13, 2):
        trial = []
        ok = True
        for c in range(N_CORES):
            p = _pack_core(core_nodes[c], cnt[core_nodes[c]], NT, k0 * 128)
            if p is None:
                ok = False
                break
            trial.append(p)
        if ok:
            packs, K0 = trial, k0
            break
    assert packs is not None, "bin packing failed"

    NPAD = NT * 128
    EPAD = NT * K0 * 128

    # --- shared weights
    We1T = np.asarray(inputs["We1"], np.float32).T  # [32, 256]
    be1 = np.asarray(inputs["be1"], np.float32)
    be2 = np.asarray(inputs["be2"], np.float32)
    bn2 = np.asarray(inputs["bn2"], np.float32)

    We2T = np.asarray(inputs["We2"], np.float32).T  # [256, 128]
    We2DR = np.concatenate([We2T[:128], We2T[128:]], axis=1)  # [128, 256]

    Wn1T = np.asarray(inputs["Wn1"], np.float32).T  # [64, 256]
    Wn1TK = np.zeros((128, 256), np.float32)
    Wn1TK[0:64] = Wn1T
    Wn1TK[64] = np.asarray(inputs["bn1"], np.float32)  # bias via ones-row

    Win1T = np.asarray(inputs["Win1"], np.float32).T  # [256, 256]
    Win1DR = np.zeros((128, 512), np.float32)
    for s in range(2):
        for i in range(2):
            Win1DR[:, 256 * s + 128 * i : 256 * s + 128 * i + 128] = \
                Win1T[128 * i : 128 * i + 128, 128 * s : 128 * s + 128]

    Wn2T = np.asarray(inputs["Wn2"], np.float32).T
    Wn2DR = np.concatenate([Wn2T[:128], Wn2T[128:]], axis=1)
    Win2T = np.asarray(inputs["Win2"], np.float32).T

    Wg2T = np.asarray(inputs["Wg2"], np.float32).T  # [16, 128]
    Wng2T = np.asarray(inputs["Wng2"], np.float32).T

    blob_shared = np.zeros((128, _BLOB_W), np.float32)

    def bput(name, arr):
        off, w = _BLOB_COLS[name]
        assert arr.shape[1] == w, (name, arr.shape)
        blob_shared[: arr.shape[0], off : off + w] = arr

    bput("We2DR", We2DR)
    bput("Wn1TK", Wn1TK)
    bput("Win1DR", Win1DR)
    bput("Wn2DR", Wn2DR)
    bput("Win2", Win2T)
    bput("WgnT", np.asarray(inputs["Wgn"], np.float32).T)
    bput("WgeT", np.asarray(inputs["Wge"], np.float32).T)
    bput("WggT", np.asarray(inputs["Wgg"], np.float32).T)
    bput("bgr", np.asarray(inputs["bg"], np.float32)[None, :])
    bput("ident2", np.eye(2, dtype=np.float32))
    bput("ident", np.eye(128, dtype=np.float32))
    bput("ones2", np.ones((1, 2), np.float32))

    slot_of_node = np.full(N_NODES, -1, np.int64)
    tile_of_node = np.full(N_NODES, -1, np.int64)
    in_maps = []
    for c in range(N_CORES):
        for t in range(NT):
            ids = packs[c][t]
            slot_of_node[ids] = t * 128 + np.arange(len(ids))
            tile_of_node[ids] = t

        # ---- edges: assign slots (grouped by receiver tile)
        eidx = np.where(edge_core == c)[0]
        et = tile_of_node[recv[eidx]]
        eorder = np.argsort(et, kind="stable")
        eidx = eidx[eorder]
        et = et[eorder]
        counts = np.bincount(et, minlength=NT)
        starts = np.concatenate([[0], np.cumsum(counts)[:-1]])
        off_in = np.arange(len(eidx)) - np.repeat(starts, counts)
        dst = et * (K0 * 128) + off_in
        assert (counts <= K0 * 128).all()

        eg_loc = graph_slot[egraph[eidx]]
        # eftM: [64, EPAD]; rows 0:32 feats, 32 ones, 33 isg0, 34 isg1,
        # rows 35:64 zero (so only rows 64:128 of the SBUF tile need memset).
        eftM = np.zeros((64, EPAD), np.float32)
        eftM[0:32, dst] = ef[eidx].T
        eftM[32, dst] = 1.0
        eftM[33, dst] = (eg_loc == 0)
        eftM[34, dst] = (eg_loc == 1)

        # one-hot selectors, chunk-major: oh2[p, ck*128 + n]
        sel = np.full(EPAD, -1, np.int64)
        sel[dst] = slot_of_node[recv[eidx]] % 128
        oh = np.zeros((EPAD, 128), np.float32)
        vmask = sel >= 0
        oh[np.where(vmask)[0], sel[vmask]] = 1.0
        oh2 = (
            oh.reshape(NT * K0, 128, 128)
            .transpose(1, 0, 2)
            .reshape(128, EPAD)
        )

        # merged e1 + e2-init stationary weights (per-core globals)
        ga, gb = core_graphs[c]
        gl = np.stack([glob[ga], glob[gb]])  # [2, 16]
        gp = gl @ Wg2T  # [2, 128]
        We1Kx = np.zeros((64, 384), np.float32)
        We1Kx[0:32, 0:256] = We1T
        We1Kx[32, 0:256] = be1
        We1Kx[32, 256:384] = be2
        We1Kx[33, 256:384] = gp[0]
        We1Kx[34, 256:384] = gp[1]

        gn = gl @ Wng2T
        gnaugK = np.zeros((128, 128), np.float32)
        gnaugK[0:2] = gn
        gnaugK[2] = bn2

        # ---- nodes
        slot_node = np.full(NPAD, -1, np.int64)
        for t in range(NT):
            ids = packs[c][t]
            slot_node[t * 128 : t * 128 + len(ids)] = ids
        valid = slot_node >= 0
        sn = np.where(valid, slot_node, 0)

        nftK = np.zeros((128, NPAD), np.float32)
        nftK[0:64][:, valid] = nf[sn[valid]].T
        nftK[64] = valid * 1.0  # ones-row pairs with the bn1 row in Wn1TK

        ng_loc = graph_slot[ngraph[sn]]
        nhotK = np.zeros((128, NPAD), np.float32)
        nhotK[0] = valid * (ng_loc == 0)
        nhotK[1] = valid * (ng_loc == 1)
        nhotK[2] = valid * 1.0

        invc2 = np.zeros((NPAD, 1), np.float32)
        invc2[valid, 0] = 1.0 / np.maximum(cnt[sn[valid]], 1)
        invc2 = invc2.reshape(NT, 128).T.copy()  # [128, NT]

        # pool weight stationaries: cols 0:2 / 128:130 carry the weights
        poolw2 = np.zeros((NPAD, 256), np.float32)
        for g in range(GPC):
            gid = core_graphs[c][g]
            m = valid & (ng_loc == g)
            poolw2[m, g] = 1.0 / max(ncnt_g[gid], 1)
            poolw2[m, 128 + g] = cnt[sn[m]] / max(ecnt_g[gid], 1)
        # trimmed pool weights [128, NT*4]:
        #   pwK[p, t*4+(0,1)] = node-pool slots, t*4+(2,3) = edge-pool slots
        pw_full = poolw2.reshape(NT, 128, 256).transpose(1, 0, 2)  # [128, NT, 256]
        pwK = np.concatenate([pw_full[:, :, 0:2], pw_full[:, :, 128:130]],
                             axis=2).reshape(128, NT * 4)

        blob = blob_shared.copy()
        blob[: gnaugK.shape[0], _BLOB_COLS["gnaugK"][0]:
             _BLOB_COLS["gnaugK"][0] + 128] = gnaugK
        blob[:2, _BLOB_COLS["globT"][0]: _BLOB_COLS["globT"][0] + 2] = 0.0
        blob[:16, _BLOB_COLS["globT"][0]: _BLOB_COLS["globT"][0] + 2] = gl.T

        m = {
            "We2DR8": We2DR.astype(npfp8),
            "Win1DR8": Win1DR.astype(npfp8),
            "Wn2DR8": Wn2DR.astype(npfp8),
            "eftM": eftM.astype(npbf16),
            "We1Kx": We1Kx.astype(npbf16),
            "wblob": blob.astype(npbf16),
            "nftK": nftK.astype(npbf16),
            "nhotK": nhotK.astype(npbf16),
            "invc2": invc2,
            "pwK": pwK.astype(npbf16),
        }
        if FP8_AGG:
            m["oh2"] = oh2.astype(npfp8)
        else:
            m["oh2"] = oh2.astype(npbf16)
        in_maps.append(m)

    return in_maps, NT, K0, [core_graphs[c] for c in range(N_CORES)]


# ---------------------------------------------------------------------------
# Device program (identical on all cores)


def _build(NT, K0):
    Relu = mybir.ActivationFunctionType.Relu
    Copy = mybir.ActivationFunctionType.Copy

    nc = bass.Bass()
    NPAD = NT * 128
    EPAD = NT * K0 * 128
    PPT = K0 // 2  # pairs per tile
    CW = K0 * 128  # eftM/oh2 cols per tile
    OH_DT = FP8 if FP8_AGG else BF16

    d_eftM = nc.dram_tensor("eftM", [64, EPAD], BF16, kind="ExternalInput")
    d_oh2 = nc.dram_tensor("oh2", [128, EPAD], OH_DT, kind="ExternalInput")
    d_We1Kx = nc.dram_tensor("We1Kx", [64, 384], BF16, kind="ExternalInput")
    d_We2DR8 = nc.dram_tensor("We2DR8", [128, 256], FP8, kind="ExternalInput")
    d_Win1DR8 = nc.dram_tensor("Win1DR8", [128, 512], FP8, kind="ExternalInput")
    d_Wn2DR8 = nc.dram_tensor("Wn2DR8", [128, 256], FP8, kind="ExternalInput")
    d_blob = nc.dram_tensor("wblob", [128, _BLOB_W], BF16, kind="ExternalInput")
    d_nftK = nc.dram_tensor("nftK", [128, NPAD], BF16, kind="ExternalInput")
    d_nhotK = nc.dram_tensor("nhotK", [128, NPAD], BF16, kind="ExternalInput")
    d_invc2 = nc.dram_tensor("invc2", [128, NT], F32, kind="ExternalInput")
    d_pwK = nc.dram_tensor("pwK", [128, NT * 4], BF16, kind="ExternalInput")
    d_out = nc.dram_tensor("out", [128, 2], F32, kind="ExternalOutput")

    with tile.TileContext(nc) as tc:
        with tc.tile_pool(name="wp", bufs=1) as wp:
            # early weights: only what the first matmuls need.
            # We1Kx lives in a 128-row tile (rows 35:128 zeroed once) so every
            # stationary in the main stream is a uniform 128x128 load — mixed
            # tiling configs keep the PE HAM clock throttled at 1.2 GHz.
            # fixed eftt buffers (manual 3-way rotation): 128-row tiles,
            # rows 64:128 zeroed once on GpSimd, DMA refills rows 0:64.
            # Tile-0's halves dispatch first: they gate the first matmul.
            eftt_bufs = []
            for k in range(3):
                b = wp.tile([128, CW], BF16, tag=f"eftt{k}")
                for q in range(64, 128, 32):
                    nc.gpsimd.memset(b[q : q + 32, :], 0.0)
                eftt_bufs.append(b)
            nc.sync.dma_start(eftt_bufs[0][0:64, 0 : CW // 2],
                              d_eftM[:, 0 : CW // 2])
            nc.sync.dma_start(eftt_bufs[0][0:64, CW // 2 : CW],
                              d_eftM[:, CW // 2 : CW])
            We1Kx = wp.tile([128, 384], BF16, tag="We1Kx")
            for q in range(64, 128, 32):
                nc.gpsimd.memset(We1Kx[q : q + 32, :], 0.0)
            nc.sync.dma_start(We1Kx[0:64, :], d_We1Kx[:])
            We2DR8 = wp.tile([128, 256], FP8, tag="We2DR8")
            nc.sync.dma_start(We2DR8[:], d_We2DR8[:])
            Win1DR8 = wp.tile([128, 512], FP8, tag="Win1DR8")
            Wn2DR8 = wp.tile([128, 256], FP8, tag="Wn2DR8")
            blob = wp.tile([128, _BLOB_W], BF16, tag="wblob")
            nftK = wp.tile([128, NPAD], BF16, tag="nftK")
            nhotK = wp.tile([128, NPAD], BF16, tag="nhotK")
            invc2 = wp.tile([128, NT], F32, tag="invc2")
            pwK = wp.tile([128, NT * 4], BF16, tag="pwK")

            def bslice(name, rows=128):
                off, w = _BLOB_COLS[name]
                return blob[0:rows, off : off + w]

            We2DR = bslice("We2DR")
            Wn1TK = bslice("Wn1TK")
            Win1DR = bslice("Win1DR")
            Wn2DR = bslice("Wn2DR")
            Win2 = bslice("Win2")
            gnaugK = bslice("gnaugK")
            WgnT = bslice("WgnT")
            WgeT = bslice("WgeT")
            WggT = bslice("WggT", rows=16)
            bgr = bslice("bgr", rows=1)
            ident2 = bslice("ident2", rows=2)
            ident = bslice("ident")
            globT = bslice("globT", rows=16)
            ones2 = bslice("ones2", rows=1)

            # deferred preload DMAs, emitted at chosen pair indices
            preloads = [
                lambda: nc.sync.dma_start(blob[:, 0:256], d_blob[:, 0:256]),
                lambda: nc.sync.dma_start(blob[:, 256:1216],
                                          d_blob[:, 256:1216]),
                lambda: nc.sync.dma_start(blob[:, 1216:], d_blob[:, 1216:]),
                lambda: nc.sync.dma_start(invc2[:], d_invc2[:]),
                lambda: nc.sync.dma_start(Win1DR8[:], d_Win1DR8[:]),
                lambda: nc.sync.dma_start(Wn2DR8[:], d_Wn2DR8[:]),
                lambda: nc.sync.dma_start(
                    nftK[:, : NPAD // 2], d_nftK[:, : NPAD // 2]),
                lambda: nc.sync.dma_start(
                    nftK[:, NPAD // 2 :], d_nftK[:, NPAD // 2 :]),
                lambda: nc.sync.dma_start(
                    nhotK[:, : NPAD // 2], d_nhotK[:, : NPAD // 2]),
                lambda: nc.sync.dma_start(
                    nhotK[:, NPAD // 2 :], d_nhotK[:, NPAD // 2 :]),
                lambda: nc.sync.dma_start(pwK[:], d_pwK[:]),
            ]

            with tc.tile_pool(name="sb", bufs=4) as sbp, \
                 tc.tile_pool(name="ppAB", bufs=2, space=bass.MemorySpace.PSUM) as ppAB, \
                 tc.tile_pool(name="ppC", bufs=2, space=bass.MemorySpace.PSUM) as ppC, \
                 tc.tile_pool(name="psAgg", bufs=1, space=bass.MemorySpace.PSUM) as psAgg, \
                 tc.tile_pool(name="psN", bufs=1, space=bass.MemorySpace.PSUM) as psN:
                ep = efp = e1p = nsb = sbp

                nodeB = psN.tile([128, 512], F32, tag="nodeB")
                pn1 = nodeB[:, 0:256]
                pn2 = nodeB[:, 256:384]

                G = NT * PPT

                # per-pair live state, indexed by global pair id
                pair_state = {}
                tile_state = {}
                node_q = {}  # emission-index -> list of (stage, tile)
                state = {"accP": None}

                def emit_pair_front_a(g):
                    t, j = divmod(g, PPT)
                    if j == 0:
                        eftt = eftt_bufs[t % 3]
                        if t > 0:
                            nc.sync.dma_start(eftt[0:64, :],
                                              d_eftM[:, t * CW:(t + 1) * CW])
                        oht = ep.tile([128, CW], OH_DT, tag="oht")
                        nc.sync.dma_start(oht[:], d_oh2[:, t * CW:(t + 1) * CW])
                        pagg = psAgg.tile([128, 384], F32, tag="pagg")
                        tile_state[t] = (eftt, oht, pagg)
                    eftt, oht, pagg = tile_state[t]

                    ptC = ppC.tile([128, 512], F32, tag="ptC")
                    epr = slice(2 * j * 128, 2 * j * 128 + 256)
                    R = 64 if t == 0 else 128
                    # e1 pre-relu, feat-major; e1bf evac starts immediately
                    nc.tensor.matmul(ptC[:, 0:256], We1Kx[0:R, 0:128],
                                     eftt[0:R, epr], start=True, stop=True)
                    nc.tensor.matmul(ptC[:, 256:512], We1Kx[0:R, 128:256],
                                     eftt[0:R, epr], start=True, stop=True)
                    e1bf = e1p.tile([128, 512], FP8, tag="e1bf")
                    nc.scalar.activation(e1bf[:], ptC[:], Relu)
                    pair_state[g] = [None, None, e1bf, oht, pagg, j, t]

                def emit_pair_front_b(g):
                    ps = pair_state[g]
                    j, t = ps[5], ps[6]
                    eftt = tile_state[t][0]
                    e0 = slice(2 * j * 128, 2 * j * 128 + 128)
                    e1s = slice((2 * j + 1) * 128, (2 * j + 1) * 128 + 128)
                    R = 64 if t == 0 else 128
                    pt = ppAB.tile([128, 1024], F32, tag="ptAB")
                    # e1 + e2init in one FD-384 matmul per chunk; group stays
                    # open until the e2 matmuls stop
                    nc.tensor.matmul(pt[:, 0:384], eftt[0:R, e0],
                                     We1Kx[0:R, 0:384], start=True, stop=False)
                    nc.tensor.matmul(pt[:, 512:896], eftt[0:R, e1s],
                                     We1Kx[0:R, 0:384], start=True, stop=False)
                    efsb = efp.tile([128, 768], FP8 if FP8_AGG else BF16,
                                    tag="efsb")
                    ps[0] = pt
                    ps[1] = efsb

                def emit_pair_e2(g):
                    pt, efsb, e1bf, oht, pagg, j, t = pair_state[g][:7]
                    e1r = e1bf[:].rearrange("p (i c) -> p i c", i=2, c=256)
                    w2r = We2DR8[:].rearrange("p (i n) -> p i n", i=2, n=128)
                    nc.tensor.matmul(pt[:, 256:384], e1r[:, :, 0:128], w2r,
                                     start=False, stop=True, perf_mode=DR)
                    nc.tensor.matmul(pt[:, 768:896], e1r[:, :, 128:256], w2r,
                                     start=False, stop=True, perf_mode=DR)
                    # whole-pair evac on DVE (one big instruction)
                    src = pt[:].rearrange(
                        "p (b c) -> p b c", b=2, c=512)[:, :, 0:384]
                    dst = efsb[:].rearrange("p (b c) -> p b c", b=2, c=384)
                    nc.vector.tensor_scalar_max(dst, src, 0.0)

                def emit_pair_agg(g):
                    ps = pair_state.pop(g)
                    pt, efsb, e1bf, oht, pagg, j, t = ps[:7]
                    first = j == 0
                    last = j == PPT - 1
                    e0 = slice(2 * j * 128, 2 * j * 128 + 128)
                    e1s = slice((2 * j + 1) * 128, (2 * j + 1) * 128 + 128)
                    if FP8_AGG:
                        efsb8 = efsb
                        nc.tensor.matmul(
                            pagg[:],
                            oht[:, 2 * j * 128:(2 * j + 2) * 128].rearrange(
                                "p (i m) -> p i m", i=2, m=128),
                            efsb8[:].rearrange("p (i n) -> p i n", i=2, n=384),
                            start=first, stop=last, perf_mode=DR,
                        )
                    else:
                        nc.tensor.matmul(pagg[:], oht[:, e0], efsb[:, 0:384],
                                         start=first, stop=False)
                        nc.tensor.matmul(pagg[:], oht[:, e1s], efsb[:, 384:768],
                                         start=False, stop=last)
                    return last, t

                def emit_node(stage, t):
                    st = tile_state[t]
                    if stage == 0:
                        # pagg -> aggsb (scaled by 1/cnt) on DVE, then DMA
                        # xbar transpose to feat-major aggT
                        pagg = st[2]
                        aggsb = nsb.tile([128, 384], BF16, tag="aggsb")
                        nc.scalar.activation(aggsb[:], pagg[:], Copy,
                                             scale=invc2[:, t : t + 1])
                        tile_state[t] = st + (aggsb,)
                    elif stage == 10:
                        # PE transpose aggsb -> aggT (feat-major)
                        aggsb = tile_state[t][3]
                        ptrT = nodeB[:, 0:384]
                        for b in range(3):
                            nc.tensor.matmul(ptrT[:, b * 128:(b + 1) * 128],
                                             aggsb[:, b * 128:(b + 1) * 128],
                                             ident, start=True, stop=True)
                        aggT = nsb.tile([128, 384], FP8, tag="aggT")
                        nc.scalar.activation(aggT[:], ptrT, Copy)
                        tile_state[t] = tile_state[t] + (aggT,)
                    elif stage == 1:
                        aggsb, aggT = tile_state[t][3], tile_state[t][4]
                        nftt = nftK[:, t * 128:(t + 1) * 128]
                        agg1r = aggT[:, 0:256].rearrange(
                            "p (i n) -> p i n", i=2, n=128)
                        for s in range(2):
                            sc = slice(s * 128, (s + 1) * 128)
                            nc.tensor.matmul(pn1[:, sc], Wn1TK[:, sc], nftt,
                                             start=True, stop=False)
                            nc.tensor.matmul(
                                pn1[:, sc],
                                Win1DR8[:, 256 * s : 256 * s + 256].rearrange(
                                    "p (i m) -> p i m", i=2, m=128),
                                agg1r, start=False, stop=True, perf_mode=DR)
                        n1bf = nsb.tile([128, 256], FP8, tag="n1bf")
                        nc.scalar.activation(n1bf[:], pn1, Relu)
                        tile_state[t] = tile_state[t] + (n1bf,)
                    elif stage == 2:
                        aggsb, aggT, n1bf = tile_state[t][3:6]
                        nht = nhotK[:, t * 128:(t + 1) * 128]
                        nc.tensor.matmul(pn2, nht, gnaugK, start=True, stop=False)
                        nc.tensor.matmul(
                            pn2,
                            n1bf[:].rearrange("p (i m) -> p i m", i=2, m=128),
                            Wn2DR8[:].rearrange("p (i n) -> p i n", i=2, n=128),
                            start=False, stop=False, perf_mode=DR)
                        nc.tensor.matmul(pn2, aggT[:, 256:384], Win2,
                                         start=False, stop=True)
                        n2bf = nsb.tile([128, 128], BF16, tag="n2bf")
                        nc.vector.tensor_scalar_max(n2bf[:], pn2, 0.0)
                        tile_state[t] = tile_state[t] + (n2bf,)
                    else:
                        aggsb, aggT, n1bf, n2bf = tile_state[t][3:7]
                        # feat-major pools: out[f, slot], FD=2
                        poolT = nodeB[:, 384:388]
                        nc.tensor.matmul(poolT[:, 0:2], n2bf[:],
                                         pwK[:, t * 4 : t * 4 + 2],
                                         start=True, stop=True)
                        nc.tensor.matmul(poolT[:, 2:4], aggsb[:, 256:384],
                                         pwK[:, t * 4 + 2 : t * 4 + 4],
                                         start=True, stop=True)
                        accP_new = nsb.tile([128, 4], F32, tag="accP")
                        if state["accP"] is None:
                            nc.vector.tensor_copy(accP_new[:], poolT)
                        else:
                            nc.vector.tensor_tensor(accP_new[:], state["accP"][:],
                                                    poolT,
                                                    op=mybir.AluOpType.add)
                        state["accP"] = accP_new
                        del tile_state[t]

                # ---------------- main pipeline ----------------
                npre = len(preloads)
                for e in range(G + 3):
                    if e >= 1 and e - 1 < G:
                        emit_pair_e2(e - 1)
                    if e < G:
                        emit_pair_front_a(e)
                    if e >= 3:
                        last, t = emit_pair_agg(e - 3)
                        if last:
                            for k, dly in ((0, 0), (10, 2), (1, 4), (2, 5),
                                           (3, 6)):
                                node_q.setdefault(e + dly, []).append((k, t))
                    if e < G:
                        emit_pair_front_b(e)
                    if 1 <= e <= npre:
                        preloads[e - 1]()
                    for stg, t in node_q.pop(e, ()):
                        emit_node(stg, t)
                # drain remaining node stages
                emax = G + 3
                for e in range(emax, emax + 6):
                    for stg, t in node_q.pop(e, ()):
                        emit_node(stg, t)

                # ----------------- final projection -----------------
                accP = state["accP"]
                accPb = nsb.tile([128, 4], BF16, tag="accPb")
                nc.vector.tensor_copy(accPb[:], accP[:])
                pout = nodeB[:, 388:390]
                nc.tensor.matmul(pout, WgnT, accPb[:, 0:2], start=True, stop=False)
                nc.tensor.matmul(pout, WgeT, accPb[:, 2:4], start=False, stop=False)
                nc.tensor.matmul(pout, WggT, globT, start=False, stop=False)
                nc.tensor.matmul(pout, bgr, ones2, start=False, stop=True)
                outsb = nsb.tile([128, 2], F32, tag="outsb")
                nc.scalar.activation(outsb[:], pout, Copy)
                nc.sync.dma_start(d_out[:], outsb[:])

    return nc


_CACHE = {}


def _get_nc(NT, K0):
    key = (NT, K0, FP8_AGG)
    if key not in _CACHE:
        _CACHE[key] = _build(NT, K0)
    return _CACHE[key]


def _run(inputs, trace=False):
    in_maps, NT, K0, core_graphs = _prepare(inputs)
    nc = _get_nc(NT, K0)
    res = run_bass_kernel_spmd(nc, in_maps, list(range(N_CORES)), trace=trace)
    out = np.zeros((N_GRAPHS, 128), np.float32)
    for c in range(N_CORES):
        r = np.asarray(res.results[c]["out"], np.float32)
        ga, gb = core_graphs[c]
        out[ga] = r[:, 0]
        out[gb] = r[:, 1]
    return out, res


def kernel(**inputs):
    out, _ = _run(inputs, trace=False)
    return out


def kernel_traced(**inputs):
    return _run(inputs, trace=True)


# revision 43
# speedup vs baseline: 1.0058x; 1.0058x over previous
"""Trainium2 Bass kernel for a 2-layer GraphNetwork (gnn_message_passing).

Strategy (final):
  - 16 graphs across 8 cores (2/core, paired big-with-small); every
    edge's receiver is core-local, so all segment reductions stay
    on-core. [16,128] outputs gathered on host.
  - Edge phase per pair of 128-edge chunks (~1.1us steady state):
      * e1 feat-major (2x FD-256, We1Kx-block stationaries) into its
        own 1-bank psum tile -> ACT relu-evac to fp8 e1bf (the split
        tile keeps the evac off the mg matmuls' dependency cone);
      * e1+e2init edge-major as one FD-384 matmul per chunk into a
        2-bank pair tile (the group stays open for e2);
      * e2 as one fp8e4 DoubleRow matmul per chunk (e1bf stationary,
        host-packed We2 pairs moving);
      * one DVE relu-evac of the whole pair ([2,384]-strided) direct
        to fp8 efsb; one DoubleRow aggregation matmul per pair
        (one-hot stationary, exact in fp8; FD 384 at 0.5 cyc/row).
    All stationaries are uniform 128x128 loads - mixed tiling configs
    re-throttle the PE HAM clock to 1.2 GHz (keep PE the busiest
    engine or the idle-window monitor oscillates K=8<->4).
  - Engine budget per pair: PE ~1.1us > DVE ~0.9 (efsb) > ACT ~0.88
    (e1bf + node evacs). Producers lead consumers by a full iteration:
    [e2(g-1), e1T(g), agg(g-2), mg(g)].
  - Node phase spread over 5 iterations per tile: pagg -> aggsb (ACT,
    1/cnt scale), PE transposes -> fp8 aggT, n1/n2 with DoubleRow
    Win1/Wn2 terms, feat-major FD-2 pools accumulated in a [128,4]
    DVE chain; bf16 final projection reads it directly.
  - Startup: 64-row eftM/We1Kx (rows 35:64 host-zeroed; rows 64:128
    memset once on GpSimd, tile 0 runs 64-row stationaries so nothing
    waits); weights in one [128, 2184] bf16 blob; node tensors are
    SBUF-resident, DMAs woven between the first tiles' input loads.
  - CoreV2/V3 codegen only accepts one semaphore wait per queue
    instruction: excess waits are split onto single-wait NOPs.
"""

import numpy as np
import ml_dtypes

import concourse.bass as bass
import concourse.tile as tile_mod
from concourse import tile
from concourse.bass_utils import run_bass_kernel_spmd
from concourse.vector_clock import ScopedClock

mybir = bass.mybir

N_NODES, N_EDGES, N_GRAPHS = 20000, 320000, 16
F_NODE, F_EDGE, F_GLOB = 64, 32, 16
N_CORES = 8
GPC = N_GRAPHS // N_CORES  # graphs per core = 2

BF16 = mybir.dt.bfloat16
F32 = mybir.dt.float32
FP8 = mybir.dt.float8e4
npbf16 = ml_dtypes.bfloat16
npfp8 = mybir.dt.np(FP8)
DR = mybir.MatmulPerfMode.DoubleRow

FP8_AGG = True  # stage-2 toggle: fp8 DoubleRow aggregation

# ---------------------------------------------------------------------------
# Workaround: CoreV2/V3 codegen rejects instructions carrying more than one
# semaphore wait (and the DMA-transpose XPOSE instruction can carry none).
# Split excess waits across single-wait NOPs issued just before on the same
# queue.
_MAX_WAITS = 1


def _split_excess_waits(nc):
    ET = mybir.EngineType
    split_engines = {ET.PE, ET.Activation, ET.DVE, ET.SP, ET.Pool}
    ctr = [0]
    for bass_bb in nc.bb_map.values():
        bb = bass_bb.bb
        out = []
        changed = False
        for inst in bb.instructions:
            si = inst.sync_info
            waits = list(si.on_wait) if (si and si.on_wait) else []
            limit = 0 if isinstance(inst, mybir.InstDmaTransposeAnt) else 1
            if len(waits) > limit and inst.engine in split_engines:
                head = waits[: len(waits) - limit]
                keep = waits[len(waits) - limit:]
                for w in head:
                    nop = mybir.InstNoOp(name=f"waitsplit-{ctr[0]}", ins=[], outs=[])
                    ctr[0] += 1
                    nop.engine = inst.engine
                    nop.sync_info = mybir.SyncInfo(on_wait=[w], on_update=[])
                    nc.register_instruction(nop, overwrite=True)
                    out.append(nop)
                inst.sync_info = mybir.SyncInfo(
                    on_wait=keep, on_update=list(si.on_update or [])
                )
                changed = True
            out.append(inst)
        if changed:
            bb.instructions = out


def _split_drain_and_barrier(self, tick_clock, wait_clock):
    nc = self.nc
    _split_excess_waits(nc)
    drain_inst = nc.sync.drain()
    wait_clock.add_sem_waits(
        drain_inst.ins, ScopedClock({None: tick_clock.global_clock})
    )
    mi = drain_inst.ins
    waits = list(mi.sync_info.on_wait) if (mi.sync_info and mi.sync_info.on_wait) else []
    if len(waits) > _MAX_WAITS:
        upd = list(mi.sync_info.on_update) if mi.sync_info.on_update else []
        mi.sync_info = mybir.SyncInfo(on_wait=waits[:_MAX_WAITS], on_update=upd)
        for i in range(_MAX_WAITS, len(waits), _MAX_WAITS):
            nop = nc.sync.nop(nofuse=True)
            nop.ins.sync_info = mybir.SyncInfo(
                on_wait=waits[i : i + _MAX_WAITS], on_update=[]
            )
    nc.all_engine_barrier()
    assert self.sems is not None
    popped = nc._tile_sem_poison_stack.pop()
    assert popped is self._sem_poison
    nc.clear_and_free_semaphores(list(self.sems.allocated().values()))
    nc.all_engine_barrier()


tile_mod.TileContext._drain_and_barrier = _split_drain_and_barrier

# ---------------------------------------------------------------------------
# The walrus invocation hardcodes --enable-ldw-opt=false; the LDWEIGHTS
# stream is a bottleneck for this kernel, so turn the optimization on.
import concourse.bass_utils as _bu

_orig_run_command = _bu.run_command


def _run_command_ldwopt(argv, **kwargs):
    argv = [
        a
        if isinstance(a, str) else a
        for a in argv
    ]
    return _orig_run_command(argv, **kwargs)


_bu.run_command = _run_command_ldwopt


# ---------------------------------------------------------------------------
# Host-side graph partitioning / layout


def _pack_core(node_ids, degs, nt, cap_e):
    order = np.argsort(-degs, kind="stable")
    tiles_n = [[] for _ in range(nt)]
    tile_ncnt = np.zeros(nt, np.int64)
    tile_ecnt = np.zeros(nt, np.int64)
    for j in order:
        cand = np.where(tile_ncnt < 128)[0]
        if len(cand) == 0:
            return None
        t = cand[np.argmin(tile_ecnt[cand])]
        tiles_n[t].append(node_ids[j])
        tile_ncnt[t] += 1
        tile_ecnt[t] += degs[j]
    if (tile_ecnt > cap_e).any():
        return None
    return [np.array(t, dtype=np.int64) for t in tiles_n]


# weight blob column layout (bf16, 128 rows)
_BLOB_COLS = {
    "We2DR": (0, 256),
    "Wn1TK": (256, 256),
    "Win1DR": (512, 512),
    "Wn2DR": (1024, 256),
    "Win2": (1280, 128),
    "gnaugK": (1408, 128),
    "WgnT": (1536, 128),
    "WgeT": (1664, 128),
    "WggT": (1792, 128),
    "bgr": (1920, 128),
    "ident2": (2048, 2),
    "globT": (2050, 2),
    "ones2": (2052, 2),
    "ident": (2056, 128),
}
_BLOB_W = 2184


def _prepare(inputs):
    nf = np.asarray(inputs["node_feats"], np.float32)
    ef = np.asarray(inputs["edge_feats"], np.float32)
    glob = np.asarray(inputs["globals_"], np.float32)
    recv = np.asarray(inputs["receivers"]).astype(np.int64)
    ngraph = np.asarray(inputs["node_graph"]).astype(np.int64)

    cnt = np.bincount(recv, minlength=N_NODES).astype(np.int64)
    egraph = ngraph[recv]
    ncnt_g = np.bincount(ngraph, minlength=N_GRAPHS)
    ecnt_g = np.bincount(egraph, minlength=N_GRAPHS)

    # pair heavy graphs with light ones to balance nodes across cores
    order = np.argsort(ncnt_g, kind="stable")
    graph_core = np.zeros(N_GRAPHS, np.int64)
    graph_slot = np.zeros(N_GRAPHS, np.int64)
    core_graphs = []
    for c in range(N_CORES):
        ga, gb = int(order[c]), int(order[N_GRAPHS - 1 - c])
        graph_core[ga] = c
        graph_slot[ga] = 0
        graph_core[gb] = c
        graph_slot[gb] = 1
        core_graphs.append((ga, gb))

    node_core = graph_core[ngraph]
    edge_core = graph_core[egraph]

    core_nodes = [np.where(node_core == c)[0] for c in range(N_CORES)]
    NT = int(max((len(cn) + 127) // 128 for cn in core_nodes))

    packs = None
    K0 = max(1, int(max(np.bincount(edge_core, minlength=N_CORES)) + NT * 128 - 1)
             // (NT * 128))
    if K0 % 2:
        K0 += 1
    for k0 in range(K0, K0 + 13, 2):
        trial = []
        ok = True
        for c in range(N_CORES):
            p = _pack_core(core_nodes[c], cnt[core_nodes[c]], NT, k0 * 128)
            if p is None:
                ok = False
                break
            trial.append(p)
        if ok:
            packs, K0 = trial, k0
            break
    assert packs is not None, "bin packing failed"

    NPAD = NT * 128
    EPAD = NT * K0 * 128

    # --- shared weights
    We1T = np.asarray(inputs["We1"], np.float32).T  # [32, 256]
    be1 = np.asarray(inputs["be1"], np.float32)
    be2 = np.asarray(inputs["be2"], np.float32)
    bn2 = np.asarray(inputs["bn2"], np.float32)

    We2T = np.asarray(inputs["We2"], np.float32).T  # [256, 128]
    We2DR = np.concatenate([We2T[:128], We2T[128:]], axis=1)  # [128, 256]

    Wn1T = np.asarray(inputs["Wn1"], np.float32).T  # [64, 256]
    Wn1TK = np.zeros((128, 256), np.float32)
    Wn1TK[0:64] = Wn1T
    Wn1TK[64] = np.asarray(inputs["bn1"], np.float32)  # bias via ones-row

    Win1T = np.asarray(inputs["Win1"], np.float32).T  # [256, 256]
    Win1DR = np.zeros((128, 512), np.float32)
    for s in range(2):
        for i in range(2):
            Win1DR[:, 256 * s + 128 * i : 256 * s + 128 * i + 128] = \
                Win1T[128 * i : 128 * i + 128, 128 * s : 128 * s + 128]

    Wn2T = np.asarray(inputs["Wn2"], np.float32).T
    Wn2DR = np.concatenate([Wn2T[:128], Wn2T[128:]], axis=1)
    Win2T = np.asarray(inputs["Win2"], np.float32).T

    Wg2T = np.asarray(inputs["Wg2"], np.float32).T  # [16, 128]
    Wng2T = np.asarray(inputs["Wng2"], np.float32).T

    blob_shared = np.zeros((128, _BLOB_W), np.float32)

    def bput(name, arr):
        off, w = _BLOB_COLS[name]
        assert arr.shape[1] == w, (name, arr.shape)
        blob_shared[: arr.shape[0], off : off + w] = arr

    bput("We2DR", We2DR)
    bput("Wn1TK", Wn1TK)
    bput("Win1DR", Win1DR)
    bput("Wn2DR", Wn2DR)
    bput("Win2", Win2T)
    bput("WgnT", np.asarray(inputs["Wgn"], np.float32).T)
    bput("WgeT", np.asarray(inputs["Wge"], np.float32).T)
    bput("WggT", np.asarray(inputs["Wgg"], np.float32).T)
    bput("bgr", np.asarray(inputs["bg"], np.float32)[None, :])
    bput("ident2", np.eye(2, dtype=np.float32))
    bput("ident", np.eye(128, dtype=np.float32))
    bput("ones2", np.ones((1, 2), np.float32))

    slot_of_node = np.full(N_NODES, -1, np.int64)
    tile_of_node = np.full(N_NODES, -1, np.int64)
    in_maps = []
    for c in range(N_CORES):
        for t in range(NT):
            ids = packs[c][t]
            slot_of_node[ids] = t * 128 + np.arange(len(ids))
            tile_of_node[ids] = t

        # ---- edges: assign slots (grouped by receiver tile)
        eidx = np.where(edge_core == c)[0]
        et = tile_of_node[recv[eidx]]
        eorder = np.argsort(et, kind="stable")
        eidx = eidx[eorder]
        et = et[eorder]
        counts = np.bincount(et, minlength=NT)
        starts = np.concatenate([[0], np.cumsum(counts)[:-1]])
        off_in = np.arange(len(eidx)) - np.repeat(starts, counts)
        dst = et * (K0 * 128) + off_in
        assert (counts <= K0 * 128).all()

        eg_loc = graph_slot[egraph[eidx]]
        # eftM: [64, EPAD]; rows 0:32 feats, 32 ones, 33 isg0, 34 isg1,
        # rows 35:64 zero (so only rows 64:128 of the SBUF tile need memset).
        eftM = np.zeros((64, EPAD), np.float32)
        eftM[0:32, dst] = ef[eidx].T
        eftM[32, dst] = 1.0
        eftM[33, dst] = (eg_loc == 0)
        eftM[34, dst] = (eg_loc == 1)

        # one-hot selectors, chunk-major: oh2[p, ck*128 + n]
        sel = np.full(EPAD, -1, np.int64)
        sel[dst] = slot_of_node[recv[eidx]] % 128
        oh = np.zeros((EPAD, 128), np.float32)
        vmask = sel >= 0
        oh[np.where(vmask)[0], sel[vmask]] = 1.0
        oh2 = (
            oh.reshape(NT * K0, 128, 128)
            .transpose(1, 0, 2)
            .reshape(128, EPAD)
        )

        # merged e1 + e2-init stationary weights (per-core globals)
        ga, gb = core_graphs[c]
        gl = np.stack([glob[ga], glob[gb]])  # [2, 16]
        gp = gl @ Wg2T  # [2, 128]
        We1Kx = np.zeros((64, 384), np.float32)
        We1Kx[0:32, 0:256] = We1T
        We1Kx[32, 0:256] = be1
        We1Kx[32, 256:384] = be2
        We1Kx[33, 256:384] = gp[0]
        We1Kx[34, 256:384] = gp[1]

        gn = gl @ Wng2T
        gnaugK = np.zeros((128, 128), np.float32)
        gnaugK[0:2] = gn
        gnaugK[2] = bn2

        # ---- nodes
        slot_node = np.full(NPAD, -1, np.int64)
        for t in range(NT):
            ids = packs[c][t]
            slot_node[t * 128 : t * 128 + len(ids)] = ids
        valid = slot_node >= 0
        sn = np.where(valid, slot_node, 0)

        nftK = np.zeros((128, NPAD), np.float32)
        nftK[0:64][:, valid] = nf[sn[valid]].T
        nftK[64] = valid * 1.0  # ones-row pairs with the bn1 row in Wn1TK

        ng_loc = graph_slot[ngraph[sn]]
        nhotK = np.zeros((128, NPAD), np.float32)
        nhotK[0] = valid * (ng_loc == 0)
        nhotK[1] = valid * (ng_loc == 1)
        nhotK[2] = valid * 1.0

        invc2 = np.zeros((NPAD, 1), np.float32)
        invc2[valid, 0] = 1.0 / np.maximum(cnt[sn[valid]], 1)
        invc2 = invc2.reshape(NT, 128).T.copy()  # [128, NT]

        # pool weight stationaries: cols 0:2 / 128:130 carry the weights
        poolw2 = np.zeros((NPAD, 256), np.float32)
        for g in range(GPC):
            gid = core_graphs[c][g]
            m = valid & (ng_loc == g)
            poolw2[m, g] = 1.0 / max(ncnt_g[gid], 1)
            poolw2[m, 128 + g] = cnt[sn[m]] / max(ecnt_g[gid], 1)
        # trimmed pool weights [128, NT*4]:
        #   pwK[p, t*4+(0,1)] = node-pool slots, t*4+(2,3) = edge-pool slots
        pw_full = poolw2.reshape(NT, 128, 256).transpose(1, 0, 2)  # [128, NT, 256]
        pwK = np.concatenate([pw_full[:, :, 0:2], pw_full[:, :, 128:130]],
                             axis=2).reshape(128, NT * 4)

        blob = blob_shared.copy()
        blob[: gnaugK.shape[0], _BLOB_COLS["gnaugK"][0]:
             _BLOB_COLS["gnaugK"][0] + 128] = gnaugK
        blob[:2, _BLOB_COLS["globT"][0]: _BLOB_COLS["globT"][0] + 2] = 0.0
        blob[:16, _BLOB_COLS["globT"][0]: _BLOB_COLS["globT"][0] + 2] = gl.T

        m = {
            "We2DR8": We2DR.astype(npfp8),
            "Win1DR8": Win1DR.astype(npfp8),
            "Wn2DR8": Wn2DR.astype(npfp8),
            "eftM": eftM.astype(npbf16),
            "We1Kx": We1Kx.astype(npbf16),
            "wblob": blob.astype(npbf16),
            "nftK": nftK.astype(npbf16),
            "nhotK": nhotK.astype(npbf16),
            "invc2": invc2,
            "pwK": pwK.astype(npbf16),
        }
        if FP8_AGG:
            m["oh2"] = oh2.astype(npfp8)
        else:
            m["oh2"] = oh2.astype(npbf16)
        in_maps.append(m)

    return in_maps, NT, K0, [core_graphs[c] for c in range(N_CORES)]


# ---------------------------------------------------------------------------
# Device program (identical on all cores)


def _build(NT, K0):
    Relu = mybir.ActivationFunctionType.Relu
    Copy = mybir.ActivationFunctionType.Copy

    nc = bass.Bass()
    NPAD = NT * 128
    EPAD = NT * K0 * 128
    PPT = K0 // 2  # pairs per tile
    CW = K0 * 128  # eftM/oh2 cols per tile
    OH_DT = FP8 if FP8_AGG else BF16

    d_eftM = nc.dram_tensor("eftM", [64, EPAD], BF16, kind="ExternalInput")
    d_oh2 = nc.dram_tensor("oh2", [128, EPAD], OH_DT, kind="ExternalInput")
    d_We1Kx = nc.dram_tensor("We1Kx", [64, 384], BF16, kind="ExternalInput")
    d_We2DR8 = nc.dram_tensor("We2DR8", [128, 256], FP8, kind="ExternalInput")
    d_Win1DR8 = nc.dram_tensor("Win1DR8", [128, 512], FP8, kind="ExternalInput")
    d_Wn2DR8 = nc.dram_tensor("Wn2DR8", [128, 256], FP8, kind="ExternalInput")
    d_blob = nc.dram_tensor("wblob", [128, _BLOB_W], BF16, kind="ExternalInput")
    d_nftK = nc.dram_tensor("nftK", [128, NPAD], BF16, kind="ExternalInput")
    d_nhotK = nc.dram_tensor("nhotK", [128, NPAD], BF16, kind="ExternalInput")
    d_invc2 = nc.dram_tensor("invc2", [128, NT], F32, kind="ExternalInput")
    d_pwK = nc.dram_tensor("pwK", [128, NT * 4], BF16, kind="ExternalInput")
    d_out = nc.dram_tensor("out", [128, 2], F32, kind="ExternalOutput")

    with tile.TileContext(nc) as tc:
        with tc.tile_pool(name="wp", bufs=1) as wp:
            # early weights: only what the first matmuls need.
            # We1Kx lives in a 128-row tile (rows 35:128 zeroed once) so every
            # stationary in the main stream is a uniform 128x128 load — mixed
            # tiling configs keep the PE HAM clock throttled at 1.2 GHz.
            # fixed eftt buffers (manual 3-way rotation): 128-row tiles,
            # rows 64:128 zeroed once on GpSimd, DMA refills rows 0:64.
            # Tile-0's halves dispatch first: they gate the first matmul.
            eftt_bufs = []
            for k in range(3):
                b = wp.tile([128, CW], BF16, tag=f"eftt{k}")
                for q in range(64, 128, 32):
                    nc.gpsimd.memset(b[q : q + 32, :], 0.0)
                eftt_bufs.append(b)
            nc.sync.dma_start(eftt_bufs[0][0:64, 0 : CW // 2],
                              d_eftM[:, 0 : CW // 2])
            nc.sync.dma_start(eftt_bufs[0][0:64, CW // 2 : CW],
                              d_eftM[:, CW // 2 : CW])
            We1Kx = wp.tile([128, 384], BF16, tag="We1Kx")
            for q in range(64, 128, 32):
                nc.gpsimd.memset(We1Kx[q : q + 32, :], 0.0)
            nc.sync.dma_start(We1Kx[0:64, :], d_We1Kx[:])
            We2DR8 = wp.tile([128, 256], FP8, tag="We2DR8")
            nc.sync.dma_start(We2DR8[:], d_We2DR8[:])
            Win1DR8 = wp.tile([128, 512], FP8, tag="Win1DR8")
            Wn2DR8 = wp.tile([128, 256], FP8, tag="Wn2DR8")
            blob = wp.tile([128, _BLOB_W], BF16, tag="wblob")
            nftK = wp.tile([128, NPAD], BF16, tag="nftK")
            nhotK = wp.tile([128, NPAD], BF16, tag="nhotK")
            invc2 = wp.tile([128, NT], F32, tag="invc2")
            pwK = wp.tile([128, NT * 4], BF16, tag="pwK")

            def bslice(name, rows=128):
                off, w = _BLOB_COLS[name]
                return blob[0:rows, off : off + w]

            We2DR = bslice("We2DR")
            Wn1TK = bslice("Wn1TK")
            Win1DR = bslice("Win1DR")
            Wn2DR = bslice("Wn2DR")
            Win2 = bslice("Win2")
            gnaugK = bslice("gnaugK")
            WgnT = bslice("WgnT")
            WgeT = bslice("WgeT")
            WggT = bslice("WggT", rows=16)
            bgr = bslice("bgr", rows=1)
            ident2 = bslice("ident2", rows=2)
            ident = bslice("ident")
            globT = bslice("globT", rows=16)
            ones2 = bslice("ones2", rows=1)

            # deferred preload DMAs, emitted at chosen pair indices
            preloads = [
                lambda: nc.sync.dma_start(Win1DR8[:], d_Win1DR8[:]),
                lambda: nc.sync.dma_start(Wn2DR8[:], d_Wn2DR8[:]),
                lambda: nc.sync.dma_start(blob[:, 0:256], d_blob[:, 0:256]),
                lambda: nc.sync.dma_start(blob[:, 256:], d_blob[:, 256:]),
                lambda: nc.sync.dma_start(invc2[:], d_invc2[:]),
                lambda: nc.sync.dma_start(
                    nftK[:, : NPAD // 2], d_nftK[:, : NPAD // 2]),
                lambda: nc.sync.dma_start(
                    nftK[:, NPAD // 2 :], d_nftK[:, NPAD // 2 :]),
                lambda: nc.sync.dma_start(
                    nhotK[:, : NPAD // 2], d_nhotK[:, : NPAD // 2]),
                lambda: nc.sync.dma_start(
                    nhotK[:, NPAD // 2 :], d_nhotK[:, NPAD // 2 :]),
                lambda: nc.sync.dma_start(pwK[:], d_pwK[:]),
            ]

            with tc.tile_pool(name="sb", bufs=4) as sbp, \
                 tc.tile_pool(name="ppAB", bufs=2, space=bass.MemorySpace.PSUM) as ppAB, \
                 tc.tile_pool(name="ppC", bufs=2, space=bass.MemorySpace.PSUM) as ppC, \
                 tc.tile_pool(name="psAgg", bufs=1, space=bass.MemorySpace.PSUM) as psAgg, \
                 tc.tile_pool(name="psN", bufs=1, space=bass.MemorySpace.PSUM) as psN:
                ep = efp = e1p = nsb = sbp

                nodeB = psN.tile([128, 512], F32, tag="nodeB")
                pn1 = nodeB[:, 0:256]
                pn2 = nodeB[:, 256:384]

                G = NT * PPT

                # per-pair live state, indexed by global pair id
                pair_state = {}
                tile_state = {}
                node_q = {}  # emission-index -> list of (stage, tile)
                state = {"accP": None}

                def emit_pair_front_a(g):
                    t, j = divmod(g, PPT)
                    if j == 0:
                        eftt = eftt_bufs[t % 3]
                        if t > 0:
                            nc.sync.dma_start(eftt[0:64, :],
                                              d_eftM[:, t * CW:(t + 1) * CW])
                        oht = ep.tile([128, CW], OH_DT, tag="oht")
                        nc.sync.dma_start(oht[:], d_oh2[:, t * CW:(t + 1) * CW])
                        pagg = psAgg.tile([128, 384], F32, tag="pagg")
                        tile_state[t] = (eftt, oht, pagg)
                    eftt, oht, pagg = tile_state[t]

                    ptC = ppC.tile([128, 512], F32, tag="ptC")
                    epr = slice(2 * j * 128, 2 * j * 128 + 256)
                    R = 64 if t == 0 else 128
                    # e1 pre-relu, feat-major; e1bf evac starts immediately
                    nc.tensor.matmul(ptC[:, 0:256], We1Kx[0:R, 0:128],
                                     eftt[0:R, epr], start=True, stop=True)
                    nc.tensor.matmul(ptC[:, 256:512], We1Kx[0:R, 128:256],
                                     eftt[0:R, epr], start=True, stop=True)
                    e1bf = e1p.tile([128, 512], FP8, tag="e1bf")
                    nc.scalar.activation(e1bf[:], ptC[:], Relu)
                    pair_state[g] = [None, None, e1bf, oht, pagg, j, t]

                def emit_pair_front_b(g):
                    ps = pair_state[g]
                    j, t = ps[5], ps[6]
                    eftt = tile_state[t][0]
                    e0 = slice(2 * j * 128, 2 * j * 128 + 128)
                    e1s = slice((2 * j + 1) * 128, (2 * j + 1) * 128 + 128)
                    R = 64 if t == 0 else 128
                    pt = ppAB.tile([128, 1024], F32, tag="ptAB")
                    # e1 + e2init in one FD-384 matmul per chunk; group stays
                    # open until the e2 matmuls stop
                    nc.tensor.matmul(pt[:, 0:384], eftt[0:R, e0],
                                     We1Kx[0:R, 0:384], start=True, stop=False)
                    nc.tensor.matmul(pt[:, 512:896], eftt[0:R, e1s],
                                     We1Kx[0:R, 0:384], start=True, stop=False)
                    efsb = efp.tile([128, 768], FP8 if FP8_AGG else BF16,
                                    tag="efsb")
                    ps[0] = pt
                    ps[1] = efsb

                def emit_pair_e2(g):
                    pt, efsb, e1bf, oht, pagg, j, t = pair_state[g][:7]
                    e1r = e1bf[:].rearrange("p (i c) -> p i c", i=2, c=256)
                    w2r = We2DR8[:].rearrange("p (i n) -> p i n", i=2, n=128)
                    nc.tensor.matmul(pt[:, 256:384], e1r[:, :, 0:128], w2r,
                                     start=False, stop=True, perf_mode=DR)
                    nc.tensor.matmul(pt[:, 768:896], e1r[:, :, 128:256], w2r,
                                     start=False, stop=True, perf_mode=DR)
                    # whole-pair evac on DVE (one big instruction)
                    src = pt[:].rearrange(
                        "p (b c) -> p b c", b=2, c=512)[:, :, 0:384]
                    dst = efsb[:].rearrange("p (b c) -> p b c", b=2, c=384)
                    nc.vector.tensor_scalar_max(dst, src, 0.0)

                def emit_pair_agg(g):
                    ps = pair_state.pop(g)
                    pt, efsb, e1bf, oht, pagg, j, t = ps[:7]
                    first = j == 0
                    last = j == PPT - 1
                    e0 = slice(2 * j * 128, 2 * j * 128 + 128)
                    e1s = slice((2 * j + 1) * 128, (2 * j + 1) * 128 + 128)
                    if FP8_AGG:
                        efsb8 = efsb
                        nc.tensor.matmul(
                            pagg[:],
                            oht[:, 2 * j * 128:(2 * j + 2) * 128].rearrange(
                                "p (i m) -> p i m", i=2, m=128),
                            efsb8[:].rearrange("p (i n) -> p i n", i=2, n=384),
                            start=first, stop=last, perf_mode=DR,
                        )
                    else:
                        nc.tensor.matmul(pagg[:], oht[:, e0], efsb[:, 0:384],
                                         start=first, stop=False)
                        nc.tensor.matmul(pagg[:], oht[:, e1s], efsb[:, 384:768],
                                         start=False, stop=last)
                    return last, t

                def emit_node(stage, t):
                    st = tile_state[t]
                    if stage == 0:
                        # pagg -> aggsb (scaled by 1/cnt) on DVE, then DMA
                        # xbar transpose to feat-major aggT
                        pagg = st[2]
                        aggsb = nsb.tile([128, 384], BF16, tag="aggsb")
                        nc.scalar.activation(aggsb[:], pagg[:], Copy,
                                             scale=invc2[:, t : t + 1])
                        tile_state[t] = st + (aggsb,)
                    elif stage == 10:
                        # PE transpose aggsb -> aggT (feat-major)
                        aggsb = tile_state[t][3]
                        ptrT = nodeB[:, 0:384]
                        for b in range(3):
                            nc.tensor.matmul(ptrT[:, b * 128:(b + 1) * 128],
                                             aggsb[:, b * 128:(b + 1) * 128],
                                             ident, start=True, stop=True)
                        aggT = nsb.tile([128, 384], FP8, tag="aggT")
                        nc.scalar.activation(aggT[:], ptrT, Copy)
                        tile_state[t] = tile_state[t] + (aggT,)
                    elif stage == 1:
                        aggsb, aggT = tile_state[t][3], tile_state[t][4]
                        nftt = nftK[:, t * 128:(t + 1) * 128]
                        agg1r = aggT[:, 0:256].rearrange(
                            "p (i n) -> p i n", i=2, n=128)
                        for s in range(2):
                            sc = slice(s * 128, (s + 1) * 128)
                            nc.tensor.matmul(pn1[:, sc], Wn1TK[:, sc], nftt,
                                             start=True, stop=False)
                            nc.tensor.matmul(
                                pn1[:, sc],
                                Win1DR8[:, 256 * s : 256 * s + 256].rearrange(
                                    "p (i m) -> p i m", i=2, m=128),
                                agg1r, start=False, stop=True, perf_mode=DR)
                        n1bf = nsb.tile([128, 256], FP8, tag="n1bf")
                        nc.scalar.activation(n1bf[:], pn1, Relu)
                        tile_state[t] = tile_state[t] + (n1bf,)
                    elif stage == 2:
                        aggsb, aggT, n1bf = tile_state[t][3:6]
                        nht = nhotK[:, t * 128:(t + 1) * 128]
                        nc.tensor.matmul(pn2, nht, gnaugK, start=True, stop=False)
                        nc.tensor.matmul(
                            pn2,
                            n1bf[:].rearrange("p (i m) -> p i m", i=2, m=128),
                            Wn2DR8[:].rearrange("p (i n) -> p i n", i=2, n=128),
                            start=False, stop=False, perf_mode=DR)
                        nc.tensor.matmul(pn2, aggT[:, 256:384], Win2,
                                         start=False, stop=True)
                        n2bf = nsb.tile([128, 128], BF16, tag="n2bf")
                        nc.vector.tensor_scalar_max(n2bf[:], pn2, 0.0)
                        tile_state[t] = tile_state[t] + (n2bf,)
                    else:
                        aggsb, aggT, n1bf, n2bf = tile_state[t][3:7]
                        # feat-major pools: out[f, slot], FD=2
                        poolT = nodeB[:, 384:388]
                        nc.tensor.matmul(poolT[:, 0:2], n2bf[:],
                                         pwK[:, t * 4 : t * 4 + 2],
                                         start=True, stop=True)
                        nc.tensor.matmul(poolT[:, 2:4], aggsb[:, 256:384],
                                         pwK[:, t * 4 + 2 : t * 4 + 4],
                                         start=True, stop=True)
                        accP_new = nsb.tile([128, 4], F32, tag="accP")
                        if state["accP"] is None:
                            nc.vector.tensor_copy(accP_new[:], poolT)
                        else:
                            nc.vector.tensor_tensor(accP_new[:], state["accP"][:],
                                                    poolT,
                                                    op=mybir.AluOpType.add)
                        state["accP"] = accP_new
                        del tile_state[t]

                # ---------------- main pipeline ----------------
                npre = len(preloads)
                for e in range(G + 3):
                    if e >= 1 and e - 1 < G:
                        emit_pair_e2(e - 1)
                    if e < G:
                        emit_pair_front_a(e)
                    if e >= 3:
                        last, t = emit_pair_agg(e - 3)
                        if last:
                            for k, dly in ((0, 0), (10, 2), (1, 4), (2, 5),
                                           (3, 6)):
                                node_q.setdefault(e + dly, []).append((k, t))
                    if e < G:
                        emit_pair_front_b(e)
                    if e < npre:
                        preloads[e]()
                    for stg, t in node_q.pop(e, ()):
                        emit_node(stg, t)
                # drain remaining node stages
                emax = G + 3
                for e in range(emax, emax + 6):
                    for stg, t in node_q.pop(e, ()):
                        emit_node(stg, t)

                # ----------------- final projection -----------------
                accP = state["accP"]
                accPb = nsb.tile([128, 4], BF16, tag="accPb")
                nc.vector.tensor_copy(accPb[:], accP[:])
                pout = nodeB[:, 388:390]
                nc.tensor.matmul(pout, WgnT, accPb[:, 0:2], start=True, stop=False)
                nc.tensor.matmul(pout, WgeT, accPb[:, 2:4], start=False, stop=False)
                nc.tensor.matmul(pout, WggT, globT, start=False, stop=False)
                nc.tensor.matmul(pout, bgr, ones2, start=False, stop=True)
                outsb = nsb.tile([128, 2], F32, tag="outsb")
                nc.scalar.activation(outsb[:], pout, Copy)
                nc.sync.dma_start(d_out[:], outsb[:])

    return nc


_CACHE = {}


def _get_nc(NT, K0):
    key = (NT, K0, FP8_AGG)
    if key not in _CACHE:
        _CACHE[key] = _build(NT, K0)
    return _CACHE[key]


def _run(inputs, trace=False):
    in_maps, NT, K0, core_graphs = _prepare(inputs)
    nc = _get_nc(NT, K0)
    res = run_bass_kernel_spmd(nc, in_maps, list(range(N_CORES)), trace=trace)
    out = np.zeros((N_GRAPHS, 128), np.float32)
    for c in range(N_CORES):
        r = np.asarray(res.results[c]["out"], np.float32)
        ga, gb = core_graphs[c]
        out[ga] = r[:, 0]
        out[gb] = r[:, 1]
    return out, res


def kernel(**inputs):
    out, _ = _run(inputs, trace=False)
    return out


def kernel_traced(**inputs):
    return _run(inputs, trace=True)
